# revision 1
# baseline (speedup 1.0000x reference)
"""Trainium2 Bass kernel for ContrastiveMaskedPatchSimilarity loss.

Computes: per-position cosine similarity along the channel axis of two
[32, 256, 64, 64] f32 tensors, then a masked mean -> scalar.

Strategy (pure data parallel over 8 NeuronCores, batch-sharded 4 each):
  - Layout on chip: [channel-chunk (128) = partitions, spatial (4096) = free].
    DMA of u/m tiles is perfectly contiguous per partition (16KB rows).
  - Elementwise products (u*m, u*u, m*m) on DVE/ACT, written as bf16.
  - Channel reduction via TensorE: product slice [128ch x 128pos] is the
    *stationary* operand (lhsT), rhs = ones[128,1] bf16 -> out[128pos, 1]
    lands position-major in PSUM, so the epilogue runs with all 128
    partitions busy.
  - Epilogue per batch: num/(sqrt(uu*mm)), fused multiply+reduce with the
    (host-pretransposed) mask, free-axis reduction -> [128, 8] partials.
  - Host: sum partials over cores, divide.
"""

import sys
from contextlib import ExitStack

import numpy as np

sys.path.insert(0, "/opt/trn_rl_repo")

import ml_dtypes  # noqa: E402

import concourse.bass as bass  # noqa: E402
import concourse.tile as tile  # noqa: E402
from concourse import bacc, mybir  # noqa: E402
from concourse.bass_utils import run_bass_kernel_spmd  # noqa: E402

B, C, H, W = 32, 256, 64, 64
NCORES = 8
BL = B // NCORES  # batches per core: 4
HWX = H * W  # 4096
ROWS = BL * C  # 1024
NPB = HWX // 128  # position blocks per batch: 32
NCHUNK = C // 128  # channel chunks: 2

F32 = mybir.dt.float32
BF16 = mybir.dt.bfloat16

_CACHED_NC = None


def build_nc():
    nc = bacc.Bacc(
        "TRN2", target_bir_lowering=False, debug=False, num_devices=NCORES
    )
    u_d = nc.dram_tensor("u", [ROWS, HWX], F32, kind="ExternalInput")
    m_d = nc.dram_tensor("m", [ROWS, HWX], F32, kind="ExternalInput")
    # mask, pre-transposed on host to [p_in (128), b*NPB + pb (128)] f32
    mk_d = nc.dram_tensor("maskf", [128, BL * NPB], F32, kind="ExternalInput")
    ones_d = nc.dram_tensor("ones", [128, 1], BF16, kind="ExternalInput")
    # out[:, 0:BL] = per-batch sum(sim*mask) partials (per partition)
    # out[:, BL:2BL] = per-batch sum(mask) partials (per partition)
    out_d = nc.dram_tensor("out", [128, 2 * BL], F32, kind="ExternalOutput")

    with tile.TileContext(nc) as tc, ExitStack() as ctx:
        const_pool = ctx.enter_context(tc.tile_pool(name="const", bufs=1))
        in_pool = ctx.enter_context(tc.tile_pool(name="inp", bufs=6))
        tmp_pool = ctx.enter_context(tc.tile_pool(name="tmp", bufs=3))
        ep_pool = ctx.enter_context(tc.tile_pool(name="ep", bufs=2))
        acc_pool = ctx.enter_context(tc.tile_pool(name="acc", bufs=1))
        psum_pool = ctx.enter_context(
            tc.tile_pool(name="psum", bufs=2, space="PSUM")
        )

        ones_t = const_pool.tile([128, 1], BF16)
        nc.sync.dma_start(ones_t[:], ones_d[:, :])
        maskf_t = const_pool.tile([128, BL * NPB], F32)
        nc.sync.dma_start(maskf_t[:], mk_d[:, :])
        acc_t = acc_pool.tile([128, 2 * BL], F32)
        # mask-only sums don't depend on tensor data: do them up front
        for b in range(BL):
            nc.vector.tensor_reduce(
                acc_t[:, BL + b : BL + b + 1],
                maskf_t[:, b * NPB : (b + 1) * NPB],
                axis=mybir.AxisListType.X,
                op=mybir.AluOpType.add,
            )

        HHX = HWX // 2  # half-tile free dim (1MB DMAs, earlier pipeline ramp)
        HPB = HHX // 128  # position blocks per half: 16
        mm_ctr = 0
        for b in range(BL):
            # PSUM cols: ch*3*NPB + stat*NPB + (h*HPB + pb)
            P = psum_pool.tile([128, NCHUNK * 3 * NPB], F32)
            for ch in range(NCHUNK):
                row0 = b * C + ch * 128
                for h in range(2):
                    csl = slice(h * HHX, (h + 1) * HHX)
                    u_t = in_pool.tile([128, HHX], F32, tag="u")
                    nc.sync.dma_start(u_t[:], u_d[row0 : row0 + 128, csl])
                    m_t = in_pool.tile([128, HHX], F32, tag="m")
                    nc.gpsimd.dma_start(m_t[:], m_d[row0 : row0 + 128, csl])

                    num_t = tmp_pool.tile([128, HHX], BF16, tag="num")
                    nc.vector.tensor_mul(num_t[:], u_t[:], m_t[:])
                    uu_t = tmp_pool.tile([128, HHX], BF16, tag="uu")
                    nc.scalar.square(uu_t[:], u_t[:])
                    mm_t = tmp_pool.tile([128, HHX], BF16, tag="mm")
                    # balance m*m between DVE (faster) and ACT so neither
                    # engine exceeds the DMA roofline
                    if mm_ctr % 3 == 0:
                        nc.vector.tensor_mul(mm_t[:], m_t[:], m_t[:])
                    else:
                        nc.scalar.square(mm_t[:], m_t[:])
                    mm_ctr += 1

                    for s, t in enumerate((num_t, uu_t, mm_t)):
                        base = ch * 3 * NPB + s * NPB + h * HPB
                        for pb in range(HPB):
                            nc.tensor.matmul(
                                P[:, base + pb : base + pb + 1],
                                t[:, pb * 128 : (pb + 1) * 128],
                                ones_t[:, :],
                                start=True,
                                stop=True,
                            )

            # epilogue for batch b (position-major [128, NPB] tiles)
            def psl(ch, s):
                c0 = ch * 3 * NPB + s * NPB
                return P[:, c0 : c0 + NPB]

            # DVE has a single PSUM read port: copy chunk-0 stats to SBUF
            # on ACT first, then add with only one PSUM operand per op.
            n0 = ep_pool.tile([128, NPB], F32, tag="n0")
            nc.scalar.copy(n0[:], psl(0, 0))
            u0 = ep_pool.tile([128, NPB], F32, tag="u0")
            nc.scalar.copy(u0[:], psl(0, 1))
            m0 = ep_pool.tile([128, NPB], F32, tag="m0")
            nc.scalar.copy(m0[:], psl(0, 2))
            numv = ep_pool.tile([128, NPB], F32, tag="numv")
            nc.vector.tensor_add(numv[:], n0[:], psl(1, 0))
            uuv = ep_pool.tile([128, NPB], F32, tag="uuv")
            nc.vector.tensor_add(uuv[:], u0[:], psl(1, 1))
            mmv = ep_pool.tile([128, NPB], F32, tag="mmv")
            nc.vector.tensor_add(mmv[:], m0[:], psl(1, 2))
            d2 = ep_pool.tile([128, NPB], F32, tag="d2")
            nc.vector.tensor_mul(d2[:], uuv[:], mmv[:])
            r = ep_pool.tile([128, NPB], F32, tag="r")
            nc.vector.reciprocal(r[:], d2[:])
            rs = ep_pool.tile([128, NPB], F32, tag="rs")
            nc.scalar.sqrt(rs[:], r[:])
            sim_t = ep_pool.tile([128, NPB], F32, tag="sim")
            nc.vector.tensor_mul(sim_t[:], numv[:], rs[:])
            simmask = ep_pool.tile([128, NPB], F32, tag="simmask")
            nc.vector.tensor_mul(
                simmask[:], sim_t[:], maskf_t[:, b * NPB : (b + 1) * NPB]
            )
            nc.vector.tensor_reduce(
                acc_t[:, b : b + 1],
                simmask[:],
                axis=mybir.AxisListType.X,
                op=mybir.AluOpType.add,
            )

        nc.sync.dma_start(out_d[:, :], acc_t[:])

    nc.compile()
    return nc


def get_nc():
    global _CACHED_NC
    if _CACHED_NC is None:
        _CACHED_NC = build_nc()
    return _CACHED_NC


def make_in_maps(unmasked, masked, latent_mask):
    ones = np.ones((128, 1), dtype=ml_dtypes.bfloat16)
    in_maps = []
    for i in range(NCORES):
        sl = slice(i * BL, (i + 1) * BL)
        u = np.ascontiguousarray(unmasked[sl]).reshape(ROWS, HWX)
        m = np.ascontiguousarray(masked[sl]).reshape(ROWS, HWX)
        mk = (
            latent_mask[sl]
            .reshape(128, 128)
            .T.astype(np.float32)
        )
        in_maps.append(
            {
                "u": u,
                "m": m,
                "maskf": np.ascontiguousarray(mk),
                "ones": ones,
            }
        )
    return in_maps


def _finalize(results):
    num = 0.0
    den = 0.0
    for res in results:
        out = np.asarray(res["out"], dtype=np.float64)
        num += out[:, :BL].sum()
        den += out[:, BL:].sum()
    return np.float32(num / den)


def kernel(unmasked_latent_tensors, masked_latent_tensors, latent_mask, **kw):
    nc = get_nc()
    in_maps = make_in_maps(
        np.asarray(unmasked_latent_tensors, dtype=np.float32),
        np.asarray(masked_latent_tensors, dtype=np.float32),
        np.asarray(latent_mask),
    )
    res = run_bass_kernel_spmd(nc, in_maps, list(range(NCORES)))
    return _finalize(res.results)


def kernel_traced(unmasked_latent_tensors, masked_latent_tensors, latent_mask):
    """Like kernel() but with NTFF tracing; returns (value, BassKernelResults)."""
    nc = get_nc()
    in_maps = make_in_maps(
        np.asarray(unmasked_latent_tensors, dtype=np.float32),
        np.asarray(masked_latent_tensors, dtype=np.float32),
        np.asarray(latent_mask),
    )
    res = run_bass_kernel_spmd(nc, in_maps, list(range(NCORES)), trace=True)
    return _finalize(res.results), res



# revision 4
# speedup vs baseline: 1.2118x; 1.2118x over previous
"""Trainium2 Bass kernel for ContrastiveMaskedPatchSimilarity loss.

Computes: per-position cosine similarity along the channel axis of two
[32, 256, 64, 64] f32 tensors, then a masked mean -> scalar.

Strategy (pure data parallel over 8 NeuronCores, batch-sharded 4 each):
  - Layout on chip: [channel-chunk (128) = partitions, spatial (4096) = free].
    DMA of u/m tiles is contiguous per partition (16KB rows); mid-stream
    tiles are the full 2 MiB row block, with quarter tiles at the very
    start (fast ramp) and very end (short tail).
  - Elementwise products (u*m, u*u, m*m) on DVE/ACT, written as bf16.
  - Channel reduction via TensorE: product slice [128ch x 128pos] is the
    *stationary* operand (lhsT), rhs = ones[128,1] bf16 -> out[128pos, 1]
    lands position-major in PSUM. The two channel chunks accumulate into
    the same PSUM column (start=ch==0, stop=ch==1).
  - Per batch: one [128, 3*NPB] PSUM->SBUF copy, DMA'd straight to DRAM.
    The cosine division, mask, and final mean run on the host (tiny).
"""

import sys
from contextlib import ExitStack

import numpy as np

sys.path.insert(0, "/opt/trn_rl_repo")

import ml_dtypes  # noqa: E402

import concourse.bass as bass  # noqa: E402
import concourse.tile as tile  # noqa: E402
from concourse import bacc, mybir  # noqa: E402
from concourse.bass_utils import run_bass_kernel_spmd  # noqa: E402

B, C, H, W = 32, 256, 64, 64
EPS = 1e-8
NCORES = 8
BL = B // NCORES  # batches per core: 4
HWX = H * W  # 4096
ROWS = BL * C  # 1024
NPB = HWX // 128  # position blocks per batch: 32
NCHUNK = C // 128  # channel chunks: 2
SOUT = 3 * NPB  # out cols per batch: 96

F32 = mybir.dt.float32
BF16 = mybir.dt.bfloat16

_CACHED_NC = None


def build_nc():
    nc = bacc.Bacc(
        "TRN2", target_bir_lowering=False, debug=False, num_devices=NCORES
    )
    u_d = nc.dram_tensor("u", [ROWS, HWX], F32, kind="ExternalInput")
    m_d = nc.dram_tensor("m", [ROWS, HWX], F32, kind="ExternalInput")
    ones_d = nc.dram_tensor("ones", [128, 1], BF16, kind="ExternalInput")
    # out col b*SOUT + s*NPB + pb, partition q = stat s of position pb*128+q
    out_d = nc.dram_tensor("out", [128, BL * SOUT], F32, kind="ExternalOutput")

    with tile.TileContext(nc) as tc, ExitStack() as ctx:
        const_pool = ctx.enter_context(tc.tile_pool(name="const", bufs=1))
        in_pool = ctx.enter_context(tc.tile_pool(name="inp", bufs=3))
        tmp_pool = ctx.enter_context(tc.tile_pool(name="tmp", bufs=2))
        st_pool = ctx.enter_context(tc.tile_pool(name="st", bufs=2))
        psum_pool = ctx.enter_context(
            tc.tile_pool(name="psum", bufs=4, space="PSUM")
        )

        ones_t = const_pool.tile([128, 1], BF16)
        nc.gpsimd.dma_start(ones_t[:], ones_d[:, :])

        # piece lists per (b, ch): column extents within the 4096 free dim.
        def pieces(b, ch):
            first = b == 0 and ch == 0
            last = b == BL - 1 and ch == NCHUNK - 1
            if first or last:
                return [(i * 1024, 1024) for i in range(4)]
            return [(0, HWX)]

        mm_ctr = 0
        for b in range(BL):
            # col ch*SOUT + s*NPB + pb (chunks side by side; added in epilogue)
            P = psum_pool.tile([128, NCHUNK * SOUT], F32)
            for ch in range(NCHUNK):
                row0 = b * C + ch * 128
                for c0, ln in pieces(b, ch):
                    csl = slice(c0, c0 + ln)
                    u_t = in_pool.tile([128, ln], F32, tag="u")
                    nc.sync.dma_start(u_t[:], u_d[row0 : row0 + 128, csl])
                    m_t = in_pool.tile([128, ln], F32, tag="m")
                    nc.sync.dma_start(m_t[:], m_d[row0 : row0 + 128, csl])

                    num_t = tmp_pool.tile([128, ln], BF16, tag="num")
                    nc.vector.tensor_mul(num_t[:], u_t[:], m_t[:])
                    uu_t = tmp_pool.tile([128, ln], BF16, tag="uu")
                    nc.scalar.square(uu_t[:], u_t[:])
                    mm_t = tmp_pool.tile([128, ln], BF16, tag="mm")
                    # alternate m*m between DVE and ACT so neither engine
                    # exceeds the DMA roofline
                    if mm_ctr % 2 == 0:
                        nc.vector.tensor_mul(mm_t[:], m_t[:], m_t[:])
                    else:
                        nc.scalar.square(mm_t[:], m_t[:])
                    mm_ctr += 1

                    pb0 = c0 // 128
                    for s, t in enumerate((num_t, uu_t, mm_t)):
                        for j in range(ln // 128):
                            col = ch * SOUT + s * NPB + pb0 + j
                            nc.tensor.matmul(
                                P[:, col : col + 1],
                                t[:, j * 128 : (j + 1) * 128],
                                ones_t[:, :],
                                start=True,
                                stop=True,
                            )

            # DVE has a single PSUM read port: stage chunk 0 via ACT first.
            c0_t = st_pool.tile([128, SOUT], F32, tag="c0")
            nc.scalar.copy(c0_t[:], P[:, :SOUT])
            st_t = st_pool.tile([128, SOUT], F32, tag="st")
            nc.vector.tensor_add(st_t[:], c0_t[:], P[:, SOUT:])
            nc.gpsimd.dma_start(
                out_d[:, b * SOUT : (b + 1) * SOUT], st_t[:]
            )

    nc.compile()
    return nc


def get_nc():
    global _CACHED_NC
    if _CACHED_NC is None:
        _CACHED_NC = build_nc()
    return _CACHED_NC


def make_in_maps(unmasked, masked):
    ones = np.ones((128, 1), dtype=ml_dtypes.bfloat16)
    in_maps = []
    for i in range(NCORES):
        sl = slice(i * BL, (i + 1) * BL)
        u = np.ascontiguousarray(unmasked[sl]).reshape(ROWS, HWX)
        m = np.ascontiguousarray(masked[sl]).reshape(ROWS, HWX)
        in_maps.append({"u": u, "m": m, "ones": ones})
    return in_maps


def _finalize(results, latent_mask):
    num = 0.0
    den = 0.0
    for i, res in enumerate(results):
        out = np.asarray(res["out"], dtype=np.float64)  # [128, BL*SOUT]
        for b in range(BL):
            blk = out[:, b * SOUT : (b + 1) * SOUT].reshape(128, 3, NPB)
            # position p = pb*128 + partition -> transpose to [pb, part]
            num_p = blk[:, 0, :].T.reshape(-1)
            uu_p = blk[:, 1, :].T.reshape(-1)
            mm_p = blk[:, 2, :].T.reshape(-1)
            den_p = np.maximum(np.sqrt(np.maximum(uu_p, 0.0)), EPS) * \
                np.maximum(np.sqrt(np.maximum(mm_p, 0.0)), EPS)
            sim = num_p / den_p
            mask = latent_mask[i * BL + b].reshape(-1) != 0
            num += sim[mask].sum()
            den += float(mask.sum())
    return np.float32(num / den)


def kernel(unmasked_latent_tensors, masked_latent_tensors, latent_mask, **kw):
    nc = get_nc()
    in_maps = make_in_maps(
        np.asarray(unmasked_latent_tensors, dtype=np.float32),
        np.asarray(masked_latent_tensors, dtype=np.float32),
    )
    res = run_bass_kernel_spmd(nc, in_maps, list(range(NCORES)))
    return _finalize(res.results, np.asarray(latent_mask))


def kernel_traced(unmasked_latent_tensors, masked_latent_tensors, latent_mask):
    """Like kernel() but with NTFF tracing; returns (value, BassKernelResults)."""
    nc = get_nc()
    in_maps = make_in_maps(
        np.asarray(unmasked_latent_tensors, dtype=np.float32),
        np.asarray(masked_latent_tensors, dtype=np.float32),
    )
    res = run_bass_kernel_spmd(nc, in_maps, list(range(NCORES)), trace=True)
    return _finalize(res.results, np.asarray(latent_mask)), res
